# revision 12
# baseline (speedup 1.0000x reference)
"""Causal self-attention (B=4, T=2048, HID=2048, NH=16, HD=128) on 8 TRN2 cores.

Tensor-parallel over heads (2 heads/core). v4 redesign vs v3 (785us):
  - den matmuls off the PE critical path: Pool-engine running slab sum S
    per (b,h,j), then ONE ones^T@S matmul (deferred into the next attn
    call's pair loop so the PE never waits on the Pool chain).
  - No Ln on ScalarE (was 33 ACT_TABLE_LOADs = 42us): rstd = rsqrt(ssq)
    seeded as exp(a*bits(ssq)+b) (Mitchell log via the fp32 bit pattern,
    Exp stays table-resident) + one DVE Newton step.
  - k is normalized on DVE (rstd_k folded into kT), so the slab exp has
    constant scale/bias and batches over PAIRS of k-blocks through a
    2-bank [128,1024] PSUM span: 160 exps instead of 320.
  - All y psum->sbuf copies on the (idle) Pool engine.
  - Host-side x1|x2 column permutation of Wq/Wk: rope/rmsnorm run on
    fully contiguous DVE views; cos/sin pre-tiled to 256 cols.
  - One shared 2-bank "big" PSUM tag serves qkv qk+v psum, the grouped
    16-transpose staging tile, and the exp pair spans; 16KB exact fit.
  - proj matmuls of the batch interleave into attn pair bubbles (2/pair).
"""

import sys

if "/opt/trn_rl_repo" not in sys.path:
    sys.path.insert(0, "/opt/trn_rl_repo")

from contextlib import ExitStack

import numpy as np

import concourse.bass as bass
import concourse.tile as tile
from concourse import bacc, mybir
from concourse.bass_utils import run_bass_kernel_spmd

F32 = mybir.dt.float32
F16 = mybir.dt.float16
I32 = mybir.dt.int32
AF = mybir.ActivationFunctionType
ALU = mybir.AluOpType
AX = mybir.AxisListType

B, T, HID = 4, 2048, 2048
NH, HD = 16, 128
N_CORES = 8
NHC = NH // N_CORES          # heads per core = 2
NC = NHC * HD                # per-core head cols = 256
TM = B * T
TBB = T // 128               # 16 t-blocks per batch
KC = HID // 128              # 16 contraction chunks
ROPE_BASE = 10000.0
EXP_BIAS = -1.25

# Mitchell rsqrt seed: rsqrt(x) ~= exp(A*float(bits(x)) + Bc)
LN2 = float(np.log(2.0))
MITCH_SIGMA = 0.0430357
RSQ_A = -0.5 * LN2 / (1 << 23)
RSQ_B = 0.5 * LN2 * (127.0 + MITCH_SIGMA)


def build_program():
    nc = bacc.Bacc("TRN2", target_bir_lowering=False, debug=False,
                   num_devices=N_CORES)

    xT = nc.dram_tensor("xT", [HID, TM], F16, kind="ExternalInput").ap()
    wqkvd = nc.dram_tensor("wqkv", [HID, 3 * NC], F16,
                           kind="ExternalInput").ap()
    wod = nc.dram_tensor("wo", [NC, HID], F16, kind="ExternalInput").ap()
    cosd = nc.dram_tensor("cos", [T, 256], F16, kind="ExternalInput").ap()
    sind = nc.dram_tensor("sin", [T, 256], F16, kind="ExternalInput").ap()
    w2d = nc.dram_tensor("w2", [128, 256], F16, kind="ExternalInput").ap()
    maskd = nc.dram_tensor("masks", [4, 128, 512], F16, kind="ExternalInput").ap()
    identd = nc.dram_tensor("ident", [128, 128], F16, kind="ExternalInput").ap()
    y = nc.dram_tensor("y", [HID, TM], F16, kind="ExternalOutput").ap()

    with tile.TileContext(nc) as tc, ExitStack() as ctx:
        consts = ctx.enter_context(tc.tile_pool(name="consts", bufs=1))
        xt_pool = ctx.enter_context(tc.tile_pool(name="xt", bufs=2))

        # first x tile, split per 128-col sub-block so the first matmul
        # only waits on a 0.5MB transfer
        xt0 = xt_pool.tile([128, KC, 512], F16, tag="xt", name="xt00")
        xt0_src = xT[:, 0:512].rearrange("(k1 k2) t -> k2 k1 t", k2=128)
        for s in range(4):
            nc.sync.dma_start(out=xt0[:, :, bass.ts(s, 128)],
                              in_=xt0_src[:, :, bass.ts(s, 128)])
        # weight upload in 4 quarters so accumulation chunk k unblocks early
        wqkv_r = wqkvd.rearrange("(k1 k2) n -> k2 k1 n", k2=128)
        wq_sb = []
        for qtr in range(4):
            wt = consts.tile([128, KC // 4, 3 * NC], F16, tag=f"wqkv{qtr}",
                             name=f"wqkv{qtr}")
            nc.sync.dma_start(out=wt, in_=wqkv_r[:, qtr * 4:(qtr + 1) * 4, :])
            wq_sb.append(wt)
        ident = consts.tile([128, 128], F16, tag="ident")
        nc.sync.dma_start(out=ident, in_=identd)
        cos_sb = consts.tile([128, TBB, 256], F16, tag="cos")
        sin_sb = consts.tile([128, TBB, 256], F16, tag="sin")
        nc.sync.dma_start(out=cos_sb,
                          in_=cosd.rearrange("(t1 t2) j -> t2 t1 j", t2=128))
        nc.sync.dma_start(out=sin_sb,
                          in_=sind.rearrange("(t1 t2) j -> t2 t1 j", t2=128))
        w2_sb = consts.tile([128, 256], F16, tag="w2")
        nc.sync.dma_start(out=w2_sb, in_=w2d)
        mask_sb = consts.tile([128, 4, 512], F16, tag="mask")
        nc.sync.dma_start(out=mask_sb, in_=maskd.rearrange("m p t -> p m t"))
        # wo not needed until the first proj job; load last
        wo_sb = consts.tile([128, NHC, HID], F16, tag="wo")
        nc.sync.dma_start(
            out=wo_sb, in_=wod.rearrange("(n1 n2) c -> n2 n1 c", n2=128))
        ones_bc = consts.tile([128, 128], F16, tag="ones")
        nc.vector.memset(ones_bc, 1.0)
        negc = consts.tile([128, 1], F32, tag="negc")
        nc.vector.memset(negc, EXP_BIAS)
        rsqb = consts.tile([128, 1], F32, tag="rsqb")
        nc.vector.memset(rsqb, RSQ_B)

        # PSUM: big 4KB*2 + outT 2KB*2 + y 2KB + den 2KB = 16KB exact
        ps_big = ctx.enter_context(tc.tile_pool(name="ps_big", bufs=2, space="PSUM"))
        ps_out = ctx.enter_context(tc.tile_pool(name="ps_out", bufs=2, space="PSUM"))
        ps_y = ctx.enter_context(tc.tile_pool(name="ps_y", bufs=1, space="PSUM"))
        ps_den = ctx.enter_context(tc.tile_pool(name="ps_den", bufs=1, space="PSUM"))

        res = ctx.enter_context(tc.tile_pool(name="res", bufs=1))
        g_pool = ctx.enter_context(tc.tile_pool(name="gp", bufs=2))
        slab_pool = ctx.enter_context(tc.tile_pool(name="slab", bufs=2))
        s_pool = ctx.enter_context(tc.tile_pool(name="sp", bufs=2))
        y_pool = ctx.enter_context(tc.tile_pool(name="yo", bufs=3))

        proj_jobs = []
        pending_den = []

        def emit_proj_job(job):
            bb, cb, tg, aT = job
            y_ps = ps_y.tile([128, 512], F32, tag="yacc",
                             name=f"yps{bb}_{cb}_{tg}")
            for n in range(NHC):
                nc.tensor.matmul(y_ps, wo_sb[:, n, bass.ts(cb, 128)],
                                 aT[:, n, bass.ds(tg * 512, 512)],
                                 start=(n == 0), stop=(n == NHC - 1))
            ysb = y_pool.tile([128, 512], F16, tag="ysb",
                              name=f"ysb{bb}_{cb}_{tg}")
            # GPSIMD can't read PSUM: split the f32->f16 drains scalar/vector
            if cb % 3 == 0:
                nc.scalar.copy(ysb, y_ps)
            else:
                nc.vector.tensor_copy(ysb, y_ps)
            nc.sync.dma_start(
                out=y[bass.ts(cb, 128), bass.ds(bb * T + tg * 512, 512)],
                in_=ysb)

        def drain_den():
            if not pending_den:
                return
            bb, j, h, outT, S, aT = pending_den.pop(0)
            den_ps = ps_den.tile([128, 512], F32, tag="den",
                                 name=f"den{bb}_{h}_{j}")
            nc.tensor.matmul(den_ps, ones_bc, S, start=True, stop=True)
            rec = s_pool.tile([128, 512], F32, tag="rec",
                              name=f"rec{bb}_{h}_{j}")
            nc.vector.reciprocal_approx_fast(rec, den_ps)
            nc.vector.tensor_mul(aT[:, h, bass.ds(j * 512, 512)], outT, rec)
            if h == NHC - 1:
                for cb in range(HID // 128):
                    proj_jobs.append((bb, cb, j, aT))

        def emit_group_transposes(pend, qkT):
            nrmq, nrmk, g = pend
            t_big = ps_big.tile([128, 1024], F32, tag="big",
                                name=f"tps{g}")
            t_all = t_big.bitcast(F16).rearrange("p (s l d) -> p s l d",
                                                 s=4, l=4)
            for sub in range(4):
                for h in range(2):
                    nc.tensor.transpose(
                        t_all[:, sub, h, :],
                        nrmq[:, sub, h, :, :].rearrange("p x d -> p (x d)"),
                        ident)
                    nc.tensor.transpose(
                        t_all[:, sub, 2 + h, :],
                        nrmk[:, sub, h, :, :].rearrange("p x d -> p (x d)"),
                        ident)
            # one strided evacuation for the whole group
            nc.scalar.copy(
                qkT[:, :, bass.ds(g * 512, 512)]
                .rearrange("p l (s d) -> p l s d", s=4),
                t_all.rearrange("p s l d -> p l s d"))

        def qkv_group(b, g, qkT, v_t, pending, xt_pre=None):
            if xt_pre is not None:
                xt = xt_pre
            else:
                xt = xt_pool.tile([128, KC, 512], F16, tag="xt")
                nc.sync.dma_start(
                    out=xt,
                    in_=xT[:, bass.ds((b * TBB + 4 * g) * 128, 512)]
                    .rearrange("(k1 k2) t -> k2 k1 t", k2=128))
            qk16 = g_pool.tile([128, 4, 512], F16, tag="qk16")
            for sub in range(4):
                tbl = 4 * g + sub
                big = ps_big.tile([128, 1024], F32, tag="big",
                                  name=f"qkv{b}_{g}_{sub}")
                qk_ps = big[:, 0:512]
                v_ps = big[:, 512:768]
                for k1 in range(KC):
                    lhs = xt[:, k1, bass.ts(sub, 128)]
                    st, sp = (k1 == 0), (k1 == KC - 1)
                    wsb = wq_sb[k1 // 4]
                    kk = k1 % 4
                    nc.tensor.matmul(qk_ps, lhs,
                                     wsb[:, kk, 0:512], start=st, stop=sp)
                    nc.tensor.matmul(v_ps, lhs,
                                     wsb[:, kk, 512:768], start=st, stop=sp)
                nc.scalar.copy(qk16[:, sub, :], qk_ps)
                nc.scalar.copy(v_t[:, tbl, :], v_ps)
            if pending[0] is not None:
                emit_group_transposes(pending[0], qkT)
                pending[0] = None

            # rope on all 4 t-blocks, contiguous x1|x2 halves
            rot = g_pool.tile([128, 4, 512], F16, tag="rot")
            tmp = g_pool.tile([128, 4, 256], F16, tag="tmp")
            x1, x2 = qk16[:, :, 0:256], qk16[:, :, 256:512]
            r1, r2 = rot[:, :, 0:256], rot[:, :, 256:512]
            ct = cos_sb[:, 4 * g:4 * g + 4, :]
            sn = sin_sb[:, 4 * g:4 * g + 4, :]
            nc.vector.tensor_mul(r1, x1, ct)
            nc.vector.tensor_mul(tmp, x2, sn)
            nc.vector.tensor_sub(r1, r1, tmp)
            nc.vector.tensor_mul(r2, x2, ct)
            nc.vector.tensor_mul(tmp, x1, sn)
            nc.vector.tensor_add(r2, r2, tmp)

            # ssq per (t-block, slot): slots = (qh0, qh1, kh0, kh1)
            sq = g_pool.tile([128, 4, 512], F16, tag="sq")
            nc.vector.tensor_mul(sq, rot, rot)
            ssq2 = g_pool.tile([128, 4, 2, 4], F32, tag="ssq2")
            nc.vector.tensor_reduce(
                ssq2.rearrange("p t x s -> p (t x s)"),
                sq.rearrange("p t (x s d) -> p (t x s) d", x=2, s=4),
                axis=AX.X, op=ALU.add)
            ssq = g_pool.tile([128, 4, 4], F32, tag="ssq")
            nc.vector.tensor_add(ssq, ssq2[:, :, 0, :], ssq2[:, :, 1, :])
            # rstd = rsqrt(ssq): Mitchell exp seed + one Newton step, no Ln
            fbits = g_pool.tile([128, 4, 4], F32, tag="fbits")
            nc.vector.tensor_copy(fbits, ssq.bitcast(I32))
            y0 = g_pool.tile([128, 4, 4], F32, tag="y0")
            nc.scalar.activation(y0, fbits, AF.Exp, scale=RSQ_A, bias=rsqb)
            t1 = g_pool.tile([128, 4, 4], F32, tag="t1")
            nc.vector.tensor_mul(t1, y0, y0)
            nc.vector.tensor_mul(t1, t1, ssq)
            nc.vector.tensor_scalar(t1, t1, -0.5, 1.5, ALU.mult, ALU.add)
            rstd = g_pool.tile([128, 4, 4], F32, tag="rstd")
            nc.vector.tensor_mul(rstd, y0, t1)

            # q_hat = rope(q) * w2 * rstd_q ; k_hat = rope(k) * rstd_k
            # layout [t, head, half, d] so each head is a contiguous 128-run
            # for the transposes; DVE caps at 3 free dims so work per half
            nrmq = g_pool.tile([128, 4, 2, 2, 64], F16, tag="nrmq")
            nrmk = g_pool.tile([128, 4, 2, 2, 64], F16, tag="nrmk")
            for xh in range(2):
                rq = (rot[:, :, bass.ds(xh * 256, 128)]
                      .rearrange("p t (s d) -> p t s d", s=2))
                w2h = (w2_sb[:, bass.ds(xh * 128, 128)]
                       .rearrange("p (s d) -> p s d", s=2))
                nc.vector.tensor_mul(
                    nrmq[:, :, :, xh, :], rq,
                    w2h[:, None].broadcast_to([128, 4, 2, 64]))
                rk = (rot[:, :, bass.ds(xh * 256 + 128, 128)]
                      .rearrange("p t (s d) -> p t s d", s=2))
                nc.vector.tensor_mul(
                    nrmk[:, :, :, xh, :], rk,
                    rstd[:, :, 2:4, None].broadcast_to([128, 4, 2, 64]))
            nc.vector.tensor_mul(
                nrmq.rearrange("p t s x d -> p t s (x d)"),
                nrmq.rearrange("p t s x d -> p t s (x d)"),
                rstd[:, :, 0:2, None].broadcast_to([128, 4, 2, 128]))
            pending[0] = (nrmq, nrmk, g)

        def attn(b, j, h, qkT, v_t, aT):
            nk = 4 * j + 4
            slab = slab_pool.tile([128, TBB, 512], F16, tag="slab",
                                  name=f"slab{b}_{h}_{j}")
            outT = ps_out.tile([128, 512], F32, tag="outT",
                               name=f"outT{b}_{h}_{j}")
            S = s_pool.tile([128, 512], F16, tag="S", name=f"S{b}_{h}_{j}")
            qrhs = qkT[:, h, bass.ds(j * 512, 512)]
            kq = qkT[:, 2 + h, :]
            for p in range(nk // 2):
                k0 = 2 * p
                big = ps_big.tile([128, 1024], F32, tag="big",
                                  name=f"st{b}_{h}_{j}_{p}")
                nc.tensor.matmul(big[:, 0:512], kq[:, bass.ts(k0, 128)],
                                 qrhs, start=True, stop=True)
                nc.tensor.matmul(big[:, 512:1024], kq[:, bass.ts(k0 + 1, 128)],
                                 qrhs, start=True, stop=True)
                if p == 1:
                    drain_den()
                nc.scalar.activation(
                    slab[:, k0:k0 + 2, :].rearrange("p k q -> p (k q)"),
                    big, AF.Exp, bias=negc, scale=1.0)
                for k in (k0, k0 + 1):
                    if k >= 4 * j:
                        nc.vector.tensor_mul(slab[:, k, :], slab[:, k, :],
                                             mask_sb[:, k - 4 * j, :])
                if p == 0:
                    nc.gpsimd.tensor_add(S, slab[:, 0, :], slab[:, 1, :])
                else:
                    nc.gpsimd.tensor_add(S, S, slab[:, k0, :])
                    nc.gpsimd.tensor_add(S, S, slab[:, k0 + 1, :])
                if p >= 1:
                    nc.tensor.matmul(outT, v_t[:, k0 - 2, bass.ds(h * HD, HD)],
                                     slab[:, k0 - 2, :],
                                     start=(p == 1), stop=False)
                    nc.tensor.matmul(outT, v_t[:, k0 - 1, bass.ds(h * HD, HD)],
                                     slab[:, k0 - 1, :],
                                     start=False, stop=False)
                for _ in range(2):
                    if proj_jobs:
                        emit_proj_job(proj_jobs.pop(0))
            if nk == 2:
                drain_den()
            nc.tensor.matmul(outT, v_t[:, nk - 2, bass.ds(h * HD, HD)],
                             slab[:, nk - 2, :], start=(nk == 2), stop=False)
            nc.tensor.matmul(outT, v_t[:, nk - 1, bass.ds(h * HD, HD)],
                             slab[:, nk - 1, :], start=False, stop=True)
            pending_den.append((b, j, h, outT, S, aT))

        for b in range(B):
            qkT = res.tile([128, 4, T], F16, name=f"qkT{b}", tag="qkT")
            v_t = res.tile([128, TBB, NC], F16, name=f"v{b}", tag="v")
            aT = res.tile([128, NHC, T], F16, name=f"aT{b}", tag=f"aT{b % 2}")
            pending = [None]
            for g in range(TBB // 4):
                qkv_group(b, g, qkT, v_t, pending,
                          xt_pre=xt0 if (b == 0 and g == 0) else None)
            if pending[0] is not None:
                emit_group_transposes(pending[0], qkT)
                pending[0] = None
            # j-major so both heads' aT columns for tg=j finish early and
            # that column's proj jobs can interleave into the remaining attn
            for j in range(T // 512):
                for h in range(NHC):
                    attn(b, j, h, qkT, v_t, aT)
        while pending_den:
            drain_den()
        while proj_jobs:
            emit_proj_job(proj_jobs.pop(0))

    nc.compile()
    return nc


_CACHE = {}


def _get_program():
    if "nc" not in _CACHE:
        _CACHE["nc"] = build_program()
    return _CACHE["nc"]


def _host_tables():
    inv = 1.0 / (ROPE_BASE ** (np.arange(0, HD, 2, dtype=np.float32) / HD))
    freqs = np.arange(T, dtype=np.float32)[:, None] * inv[None, :]
    cos = np.tile(np.cos(freqs), (1, 4)).astype(np.float16)
    sin = np.tile(np.sin(freqs), (1, 4)).astype(np.float16)
    m = np.zeros((4, 128, 512), dtype=np.float16)
    s_idx = np.arange(128)[:, None]
    t_idx = np.arange(512)[None, :]
    for off in range(4):
        m[off] = ((off * 128 + s_idx) <= t_idx).astype(np.float16)
    return cos, sin, m


def kernel(x, Wq, Wk, Wv, Wo, q_rms_w, k_rms_w, **_):
    nc = _get_program()
    cos, sin, masks = _host_tables()
    xT = np.ascontiguousarray(
        np.asarray(x, dtype=np.float32).reshape(TM, HID).T).astype(np.float16)
    w2 = (np.asarray(q_rms_w, dtype=np.float32)
          * np.asarray(k_rms_w, dtype=np.float32) * np.sqrt(HD))
    # permuted q-col layout [half, head, d]
    w2p = np.stack([np.stack([w2[0:64]] * NHC), np.stack([w2[64:128]] * NHC)])
    w2_b = np.ascontiguousarray(
        np.broadcast_to(w2p.reshape(-1)[None, :], (128, 256))).astype(np.float16)
    ident_h = np.eye(128, dtype=np.float16)

    in_maps = []
    for c in range(N_CORES):
        cols = slice(c * NC, (c + 1) * NC)
        # qk columns permuted to [half(x1|x2), tensor(q|k), head, d]
        qk = np.stack([np.asarray(Wq)[:, cols], np.asarray(Wk)[:, cols]], 1)
        qk = qk.reshape(HID, 2, NHC, 2, 64).transpose(0, 3, 1, 2, 4)
        qk = np.ascontiguousarray(qk.reshape(HID, 2 * NC))
        in_maps.append({
            "xT": xT,
            "wqkv": np.ascontiguousarray(
                np.concatenate([qk, np.asarray(Wv)[:, cols]], axis=1)
            ).astype(np.float16),
            "wo": np.ascontiguousarray(Wo[cols, :]).astype(np.float16),
            "cos": cos, "sin": sin, "w2": w2_b, "masks": masks,
            "ident": ident_h,
        })

    res = run_bass_kernel_spmd(nc, in_maps, list(range(N_CORES)))
    out = res.results[0]["y"].astype(np.float32)
    for c in range(1, N_CORES):
        out += res.results[c]["y"]
    return np.ascontiguousarray(out.T).reshape(B, T, HID).astype(np.float32)


# revision 16
# speedup vs baseline: 1.2039x; 1.2039x over previous
"""Causal self-attention (B=4, T=2048, HID=2048, NH=16, HD=128) on 8 TRN2 cores.

Tensor-parallel over heads (2 heads/core). v5: keep the PE stream
continuous (stalls also drop the PE out of its max p-state, so every
bubble costs ~2x). den stays on the PE as per-k ones^T@slab accumulation
(v4's Pool-chain den stalled the in-order PE queue for ~150us). vs v3:
  - No Ln on ScalarE (was 33 ACT_TABLE_LOADs = 42us): rstd = rsqrt(ssq)
    seeded as exp(a*bits(ssq)+b) (Mitchell log via the fp32 bit pattern,
    Exp stays table-resident) + one DVE Newton step.
  - k normalized on DVE (rstd_k folded into kT), so the slab exp has
    constant scale/bias and no cross-engine scale-AP dependency.
  - Host-side x1|x2 column permutation of Wq/Wk: rope/rmsnorm run on
    contiguous DVE views; cos/sin pre-tiled to 256 cols.
  - Transposes drain one sub-block at a time through a queue popped
    between qkv sub-blocks and attn k-iters: the psum ring never forces
    a PE wait on the scalar evacuation copy.
  - PV delayed by TWO k-blocks so exp+mask latency never stalls it.
  - y psum double-buffered; y drains split scalar(1/3)/vector(2/3).
  - First x tile and wqkv upload split so the first matmul starts ~2.5us
    after launch instead of ~11us.
"""

import sys

if "/opt/trn_rl_repo" not in sys.path:
    sys.path.insert(0, "/opt/trn_rl_repo")

from contextlib import ExitStack

import numpy as np

import concourse.bass as bass
import concourse.tile as tile
from concourse import bacc, mybir
from concourse.bass_utils import run_bass_kernel_spmd

F32 = mybir.dt.float32
F16 = mybir.dt.float16
I32 = mybir.dt.int32
AF = mybir.ActivationFunctionType
ALU = mybir.AluOpType
AX = mybir.AxisListType

B, T, HID = 4, 2048, 2048
NH, HD = 16, 128
N_CORES = 8
NHC = NH // N_CORES          # heads per core = 2
NC = NHC * HD                # per-core head cols = 256
TM = B * T
TBB = T // 128               # 16 t-blocks per batch
KC = HID // 128              # 16 contraction chunks
ROPE_BASE = 10000.0
EXP_BIAS = -1.25

# Mitchell rsqrt seed: rsqrt(x) ~= exp(A*float(bits(x)) + Bc)
LN2 = float(np.log(2.0))
MITCH_SIGMA = 0.0430357
RSQ_A = -0.5 * LN2 / (1 << 23)
RSQ_B = 0.5 * LN2 * (127.0 + MITCH_SIGMA)


def build_program():
    nc = bacc.Bacc("TRN2", target_bir_lowering=False, debug=False,
                   num_devices=N_CORES)

    xT = nc.dram_tensor("xT", [HID, TM], F16, kind="ExternalInput").ap()
    wqkvd = nc.dram_tensor("wqkv", [HID, 3 * NC], F16,
                           kind="ExternalInput").ap()
    wod = nc.dram_tensor("wo", [NC, HID], F16, kind="ExternalInput").ap()
    cosd = nc.dram_tensor("cos", [T, 256], F16, kind="ExternalInput").ap()
    sind = nc.dram_tensor("sin", [T, 256], F16, kind="ExternalInput").ap()
    w2d = nc.dram_tensor("w2", [128, 256], F16, kind="ExternalInput").ap()
    maskd = nc.dram_tensor("masks", [4, 128, 512], F16, kind="ExternalInput").ap()
    identd = nc.dram_tensor("ident", [128, 128], F16, kind="ExternalInput").ap()
    y = nc.dram_tensor("y", [HID, TM], F16, kind="ExternalOutput").ap()

    with tile.TileContext(nc) as tc, ExitStack() as ctx:
        consts = ctx.enter_context(tc.tile_pool(name="consts", bufs=1))
        xt_pool = ctx.enter_context(tc.tile_pool(name="xt", bufs=2))

        # first x tile, split per 128-col sub-block so the first matmul
        # only waits on a 0.5MB transfer
        xt0 = xt_pool.tile([128, KC, 512], F16, tag="xt", name="xt00")
        xt0_src = xT[:, 0:512].rearrange("(k1 k2) t -> k2 k1 t", k2=128)
        for s in range(4):
            nc.sync.dma_start(out=xt0[:, :, bass.ts(s, 128)],
                              in_=xt0_src[:, :, bass.ts(s, 128)])
        # weight upload in 4 quarters so accumulation chunk k unblocks early
        wqkv_r = wqkvd.rearrange("(k1 k2) n -> k2 k1 n", k2=128)
        wq_sb = []
        for qtr in range(4):
            wt = consts.tile([128, KC // 4, 3 * NC], F16, tag=f"wqkv{qtr}",
                             name=f"wqkv{qtr}")
            nc.sync.dma_start(out=wt, in_=wqkv_r[:, qtr * 4:(qtr + 1) * 4, :])
            wq_sb.append(wt)
        ident = consts.tile([128, 128], F16, tag="ident")
        nc.sync.dma_start(out=ident, in_=identd)
        cos_sb = consts.tile([128, TBB, 256], F16, tag="cos")
        sin_sb = consts.tile([128, TBB, 256], F16, tag="sin")
        nc.sync.dma_start(out=cos_sb,
                          in_=cosd.rearrange("(t1 t2) j -> t2 t1 j", t2=128))
        nc.sync.dma_start(out=sin_sb,
                          in_=sind.rearrange("(t1 t2) j -> t2 t1 j", t2=128))
        w2_sb = consts.tile([128, 256], F16, tag="w2")
        nc.sync.dma_start(out=w2_sb, in_=w2d)
        mask_sb = consts.tile([128, 4, 512], F16, tag="mask")
        nc.sync.dma_start(out=mask_sb, in_=maskd.rearrange("m p t -> p m t"))
        # wo not needed until the first proj job; load last
        wo_sb = consts.tile([128, NHC, HID], F16, tag="wo")
        nc.sync.dma_start(
            out=wo_sb, in_=wod.rearrange("(n1 n2) c -> n2 n1 c", n2=128))
        ones_bc = consts.tile([128, 128], F16, tag="ones")
        nc.vector.memset(ones_bc, 1.0)
        negc = consts.tile([128, 1], F32, tag="negc")
        nc.vector.memset(negc, EXP_BIAS)
        rsqb = consts.tile([128, 1], F32, tag="rsqb")
        nc.vector.memset(rsqb, RSQ_B)

        # PSUM banks (8): qk*2 + (st|v shared, phase-disjoint)*2 + acc*2
        # (outT+den ring) + tr*1 + y*1
        ps_qk = ctx.enter_context(tc.tile_pool(name="ps_qk", bufs=2, space="PSUM"))
        ps_sv = ctx.enter_context(tc.tile_pool(name="ps_sv", bufs=2, space="PSUM"))
        ps_tr = ctx.enter_context(tc.tile_pool(name="ps_tr", bufs=1, space="PSUM"))
        ps_acc = ctx.enter_context(tc.tile_pool(name="ps_acc", bufs=2, space="PSUM"))
        ps_y = ctx.enter_context(tc.tile_pool(name="ps_y", bufs=1, space="PSUM"))

        res = ctx.enter_context(tc.tile_pool(name="res", bufs=1))
        g_pool = ctx.enter_context(tc.tile_pool(name="gp", bufs=2))
        slab_pool = ctx.enter_context(tc.tile_pool(name="slab", bufs=2))
        rec_pool = ctx.enter_context(tc.tile_pool(name="rc", bufs=2))
        y_pool = ctx.enter_context(tc.tile_pool(name="yo", bufs=3))

        proj_jobs = []
        tr_jobs = []

        def emit_proj_job(job):
            bb, cb, tg, aT = job
            y_ps = ps_y.tile([128, 512], F32, tag="yacc",
                             name=f"yps{bb}_{cb}_{tg}")
            for n in range(NHC):
                nc.tensor.matmul(y_ps, wo_sb[:, n, bass.ts(cb, 128)],
                                 aT[:, n, bass.ds(tg * 512, 512)],
                                 start=(n == 0), stop=(n == NHC - 1))
            ysb = y_pool.tile([128, 512], F16, tag="ysb",
                              name=f"ysb{bb}_{cb}_{tg}")
            # split the f32->f16 psum drains between scalar and vector
            if cb % 3 == 0:
                nc.scalar.copy(ysb, y_ps)
            else:
                nc.vector.tensor_copy(ysb, y_ps)
            nc.sync.dma_start(
                out=y[bass.ts(cb, 128), bass.ds(bb * T + tg * 512, 512)],
                in_=ysb)

        def emit_tr_sub(job):
            nrmq, nrmk, g, sub, qkT = job
            tbl = 4 * g + sub
            t_ps = ps_tr.tile([128, 4, 128], F16, tag="tr",
                              name=f"tps{tbl}")
            for h in range(2):
                nc.tensor.transpose(
                    t_ps[:, h, :],
                    nrmq[:, sub, h, :, :].rearrange("p x d -> p (x d)"),
                    ident)
                nc.tensor.transpose(
                    t_ps[:, 2 + h, :],
                    nrmk[:, sub, h, :, :].rearrange("p x d -> p (x d)"),
                    ident)
            nc.scalar.copy(qkT[:, :, bass.ds(tbl * 128, 128)], t_ps)

        def qkv_group(b, g, qkT, v_t, xt_pre=None):
            if xt_pre is not None:
                xt = xt_pre
            else:
                xt = xt_pool.tile([128, KC, 512], F16, tag="xt")
                nc.sync.dma_start(
                    out=xt,
                    in_=xT[:, bass.ds((b * TBB + 4 * g) * 128, 512)]
                    .rearrange("(k1 k2) t -> k2 k1 t", k2=128))
            qk16 = g_pool.tile([128, 4, 512], F16, tag="qk16")
            for sub in range(4):
                tbl = 4 * g + sub
                qk_ps = ps_qk.tile([128, 512], F32, tag="qk",
                                   name=f"qkv{b}_{g}_{sub}")
                v_ps = ps_sv.tile([128, 256], F32, tag="sv",
                                  name=f"v{b}_{g}_{sub}")
                for k1 in range(KC):
                    lhs = xt[:, k1, bass.ts(sub, 128)]
                    st, sp = (k1 == 0), (k1 == KC - 1)
                    wsb = wq_sb[k1 // 4]
                    kk = k1 % 4
                    nc.tensor.matmul(qk_ps, lhs,
                                     wsb[:, kk, 0:512], start=st, stop=sp)
                    nc.tensor.matmul(v_ps, lhs,
                                     wsb[:, kk, 512:768], start=st, stop=sp)
                nc.scalar.copy(qk16[:, sub, :], qk_ps)
                nc.scalar.copy(v_t[:, tbl, :], v_ps)
                if tr_jobs:
                    emit_tr_sub(tr_jobs.pop(0))

            # rope on all 4 t-blocks, contiguous x1|x2 halves
            rot = g_pool.tile([128, 4, 512], F16, tag="rot")
            tmp = g_pool.tile([128, 4, 256], F16, tag="tmp")
            x1, x2 = qk16[:, :, 0:256], qk16[:, :, 256:512]
            r1, r2 = rot[:, :, 0:256], rot[:, :, 256:512]
            ct = cos_sb[:, 4 * g:4 * g + 4, :]
            sn = sin_sb[:, 4 * g:4 * g + 4, :]
            nc.vector.tensor_mul(r1, x1, ct)
            nc.vector.tensor_mul(tmp, x2, sn)
            nc.vector.tensor_sub(r1, r1, tmp)
            nc.vector.tensor_mul(r2, x2, ct)
            nc.vector.tensor_mul(tmp, x1, sn)
            nc.vector.tensor_add(r2, r2, tmp)

            # ssq per (t-block, slot): slots = (qh0, qh1, kh0, kh1)
            sq = g_pool.tile([128, 4, 512], F16, tag="sq")
            nc.vector.tensor_mul(sq, rot, rot)
            ssq2 = g_pool.tile([128, 4, 2, 4], F32, tag="ssq2")
            nc.vector.tensor_reduce(
                ssq2.rearrange("p t x s -> p (t x s)"),
                sq.rearrange("p t (x s d) -> p (t x s) d", x=2, s=4),
                axis=AX.X, op=ALU.add)
            ssq = g_pool.tile([128, 4, 4], F32, tag="ssq")
            nc.vector.tensor_add(ssq, ssq2[:, :, 0, :], ssq2[:, :, 1, :])
            # rstd = rsqrt(ssq): Mitchell exp seed + one Newton step, no Ln
            fbits = g_pool.tile([128, 4, 4], F32, tag="fbits")
            nc.vector.tensor_copy(fbits, ssq.bitcast(I32))
            y0 = g_pool.tile([128, 4, 4], F32, tag="y0")
            nc.scalar.activation(y0, fbits, AF.Exp, scale=RSQ_A, bias=rsqb)
            t1 = g_pool.tile([128, 4, 4], F32, tag="t1")
            nc.vector.tensor_mul(t1, y0, y0)
            nc.vector.tensor_mul(t1, t1, ssq)
            nc.vector.tensor_scalar(t1, t1, -0.5, 1.5, ALU.mult, ALU.add)
            rstd = g_pool.tile([128, 4, 4], F32, tag="rstd")
            nc.vector.tensor_mul(rstd, y0, t1)

            # q_hat = rope(q) * w2 * rstd_q ; k_hat = rope(k) * rstd_k
            # layout [t, head, half, d]: each head a contiguous 128-run for
            # the transposes; DVE caps at 3 free dims so work per half
            nrmq = g_pool.tile([128, 4, 2, 2, 64], F16, tag="nrmq")
            nrmk = g_pool.tile([128, 4, 2, 2, 64], F16, tag="nrmk")
            for xh in range(2):
                rq = (rot[:, :, bass.ds(xh * 256, 128)]
                      .rearrange("p t (s d) -> p t s d", s=2))
                w2h = (w2_sb[:, bass.ds(xh * 128, 128)]
                       .rearrange("p (s d) -> p s d", s=2))
                nc.vector.tensor_mul(
                    nrmq[:, :, :, xh, :], rq,
                    w2h[:, None].broadcast_to([128, 4, 2, 64]))
                rk = (rot[:, :, bass.ds(xh * 256 + 128, 128)]
                      .rearrange("p t (s d) -> p t s d", s=2))
                nc.vector.tensor_mul(
                    nrmk[:, :, :, xh, :], rk,
                    rstd[:, :, 2:4, None].broadcast_to([128, 4, 2, 64]))
            nc.vector.tensor_mul(
                nrmq.rearrange("p t s x d -> p t s (x d)"),
                nrmq.rearrange("p t s x d -> p t s (x d)"),
                rstd[:, :, 0:2, None].broadcast_to([128, 4, 2, 128]))
            for sub in range(4):
                tr_jobs.append((nrmq, nrmk, g, sub, qkT))

        def attn(b, j, h, qkT, v_t, aT):
            nk = 4 * j + 4
            slab = slab_pool.tile([128, TBB, 512], F16, tag="slab",
                                  name=f"slab{b}_{h}_{j}")
            outT = ps_acc.tile([128, 512], F32, tag="acc",
                               name=f"outT{b}_{h}_{j}")
            den = ps_acc.tile([128, 512], F32, tag="acc",
                              name=f"den{b}_{h}_{j}")
            qrhs = qkT[:, h, bass.ds(j * 512, 512)]
            kq = qkT[:, 2 + h, :]
            for k in range(nk):
                st_ps = ps_sv.tile([128, 512], F32, tag="sv",
                                   name=f"st{b}_{h}_{j}_{k}")
                nc.tensor.matmul(st_ps, kq[:, bass.ts(k, 128)], qrhs,
                                 start=True, stop=True)
                nc.scalar.activation(slab[:, k, :], st_ps, AF.Exp,
                                     bias=negc, scale=1.0)
                if k >= 4 * j:
                    nc.vector.tensor_mul(slab[:, k, :], slab[:, k, :],
                                         mask_sb[:, k - 4 * j, :])
                nc.tensor.matmul(den, ones_bc, slab[:, k, :],
                                 start=(k == 0), stop=(k == nk - 1))
                if k >= 2:
                    nc.tensor.matmul(outT, v_t[:, k - 2, bass.ds(h * HD, HD)],
                                     slab[:, k - 2, :],
                                     start=(k == 2), stop=False)
                if tr_jobs:
                    emit_tr_sub(tr_jobs.pop(0))
                elif proj_jobs:
                    emit_proj_job(proj_jobs.pop(0))
            nc.tensor.matmul(outT, v_t[:, nk - 2, bass.ds(h * HD, HD)],
                             slab[:, nk - 2, :], start=False, stop=False)
            nc.tensor.matmul(outT, v_t[:, nk - 1, bass.ds(h * HD, HD)],
                             slab[:, nk - 1, :], start=False, stop=True)
            rec = rec_pool.tile([128, 512], F32, tag="rec",
                                name=f"rec{b}_{h}_{j}")
            nc.vector.reciprocal_approx_fast(rec, den)
            nc.vector.tensor_mul(aT[:, h, bass.ds(j * 512, 512)], outT, rec)
            if h == NHC - 1:
                for cb in range(HID // 128):
                    proj_jobs.append((b, cb, j, aT))

        for b in range(B):
            qkT = res.tile([128, 4, T], F16, name=f"qkT{b}", tag="qkT")
            v_t = res.tile([128, TBB, NC], F16, name=f"v{b}", tag="v")
            aT = res.tile([128, NHC, T], F16, name=f"aT{b}", tag=f"aT{b % 2}")
            for g in range(TBB // 4):
                qkv_group(b, g, qkT, v_t,
                          xt_pre=xt0 if (b == 0 and g == 0) else None)
            # j-major so both heads' aT columns for tg=j finish early and
            # that column's proj jobs can interleave into the remaining attn
            for j in range(T // 512):
                for h in range(NHC):
                    attn(b, j, h, qkT, v_t, aT)
        while tr_jobs:
            emit_tr_sub(tr_jobs.pop(0))
        while proj_jobs:
            emit_proj_job(proj_jobs.pop(0))

    nc.compile()
    return nc


_CACHE = {}


def _get_program():
    if "nc" not in _CACHE:
        _CACHE["nc"] = build_program()
    return _CACHE["nc"]


def _host_tables():
    inv = 1.0 / (ROPE_BASE ** (np.arange(0, HD, 2, dtype=np.float32) / HD))
    freqs = np.arange(T, dtype=np.float32)[:, None] * inv[None, :]
    cos = np.tile(np.cos(freqs), (1, 4)).astype(np.float16)
    sin = np.tile(np.sin(freqs), (1, 4)).astype(np.float16)
    m = np.zeros((4, 128, 512), dtype=np.float16)
    s_idx = np.arange(128)[:, None]
    t_idx = np.arange(512)[None, :]
    for off in range(4):
        m[off] = ((off * 128 + s_idx) <= t_idx).astype(np.float16)
    return cos, sin, m


def kernel(x, Wq, Wk, Wv, Wo, q_rms_w, k_rms_w, **_):
    nc = _get_program()
    cos, sin, masks = _host_tables()
    xT = np.ascontiguousarray(
        np.asarray(x, dtype=np.float32).reshape(TM, HID).T).astype(np.float16)
    w2 = (np.asarray(q_rms_w, dtype=np.float32)
          * np.asarray(k_rms_w, dtype=np.float32) * np.sqrt(HD))
    # permuted q-col layout [half, head, d]
    w2p = np.stack([np.stack([w2[0:64]] * NHC), np.stack([w2[64:128]] * NHC)])
    w2_b = np.ascontiguousarray(
        np.broadcast_to(w2p.reshape(-1)[None, :], (128, 256))).astype(np.float16)
    ident_h = np.eye(128, dtype=np.float16)

    in_maps = []
    for c in range(N_CORES):
        cols = slice(c * NC, (c + 1) * NC)
        # qk columns permuted to [half(x1|x2), tensor(q|k), head, d]
        qk = np.stack([np.asarray(Wq)[:, cols], np.asarray(Wk)[:, cols]], 1)
        qk = qk.reshape(HID, 2, NHC, 2, 64).transpose(0, 3, 1, 2, 4)
        qk = np.ascontiguousarray(qk.reshape(HID, 2 * NC))
        in_maps.append({
            "xT": xT,
            "wqkv": np.ascontiguousarray(
                np.concatenate([qk, np.asarray(Wv)[:, cols]], axis=1)
            ).astype(np.float16),
            "wo": np.ascontiguousarray(Wo[cols, :]).astype(np.float16),
            "cos": cos, "sin": sin, "w2": w2_b, "masks": masks,
            "ident": ident_h,
        })

    res = run_bass_kernel_spmd(nc, in_maps, list(range(N_CORES)))
    out = res.results[0]["y"].astype(np.float32)
    for c in range(1, N_CORES):
        out += res.results[c]["y"]
    return np.ascontiguousarray(out.T).reshape(B, T, HID).astype(np.float32)
